# revision 51
# baseline (speedup 1.0000x reference)
"""Trainium2 Bass kernel for AdvancedTransformerEncoderBlock.

Sharding: token-parallel across 8 cores (B=2 x 4 seq chunks of 512), each core
recomputes a 256-token K/V halo -> zero collectives. Per-core work:
  LN1 -> QKV (q/k transposed layout, v natural) -> RoPE -> local causal
  attention (window 256) -> out-proj + residual -> LN2 -> SwiGLU MLP + residual.
LN scale/shift params are folded into the weights/biases on the host.

All matmul operands are bf16 (halves weight DMA, full-rate PE at any tile
size); PSUM accumulation stays fp32, residual stream stays fp32.
RoPE rotate-half runs as a PE permutation matmul; the attention band mask and
the v/down biases are folded into PSUM via identity / ones-row matmuls, so
softmax needs no DVE mask add and exp reads PSUM directly.
"""

import numpy as np

B, S, D, F, H, HD = 2, 2048, 1024, 4096, 16, 64
WIN = 256
NCORES = 8
CH = 4           # chunks per batch
CS = S // CH     # 512 tokens per chunk (queries)
HT = CS + WIN    # 768 tokens incl. halo (keys/values)
NQT = CS // 128  # 4 query tiles
NKT = HT // 128  # 6 key tiles
EPS = 1e-5
NEG = -1e9


def build_program():
    import concourse.bass as bass
    import concourse.bacc as bacc_mod
    import concourse.tile as tile
    import concourse.mybir as mybir
    from concourse.masks import make_identity
    from contextlib import ExitStack

    dt = mybir.dt
    f32, bf16 = dt.float32, dt.bfloat16
    AF = mybir.ActivationFunctionType
    OP = mybir.AluOpType

    nc = bacc_mod.Bacc()
    P = lambda name, shape: nc.declare_dram_parameter(name, list(shape), f32, isOutput=False)
    Pb = lambda name, shape: nc.declare_dram_parameter(name, list(shape), bf16, isOutput=False)

    xh_d = Pb("xh", (HT, D))
    wq_d = Pb("wq", (8, 128, 8, 128))      # [mt][p=k][kt][m]
    wk_d = Pb("wk", (8, 128, 8, 128))
    wv_d = Pb("wv", (2, 8, 128, 512))      # [ch][kt][p][n] v-weight halves
    wo_d = Pb("wo", (8, 128, D))
    wg_d = Pb("wg", (32, 128, 8, 128))
    wu_d = Pb("wu", (32, 128, 8, 128))
    wd_d = Pb("wd", (32, 128, D))
    bv_d = Pb("bv", (1, D))
    bd_d = Pb("bd", (1, D))
    # packed constants: one f32 blob (biases, host pre-transposed) and one
    # bf16 blob (rope tables, masks, rotate-half permutation) -> 2 DMAs
    cbf_d = P("cbf", (128, 80))
    cbb_d = Pb("cbb", (128, 4224))
    out_d = nc.declare_dram_parameter("out", [CS, D], f32, isOutput=True)

    with tile.TileContext(nc) as tc, ExitStack() as top:
        const = top.enter_context(tc.tile_pool(name="const", bufs=1))

        # x tiles first: their DMAs head the queue so LN/transposes start early
        x_pool = top.enter_context(tc.tile_pool(name="x", bufs=6))
        x_tiles = []
        for tt in range(6):
            xt = x_pool.tile([128, D], bf16, tag="xt")
            # split dispatch SP/Pool: Pool's SWDGE path bypasses the HWDGE
            eng = nc.sync if tt % 2 == 0 else nc.gpsimd
            if tt == 0:
                # halves so bn_stats on cols 0-511 starts as soon as possible
                eng.dma_start(out=xt[:, 0:512], in_=xh_d[0:128, 0:512])
                eng.dma_start(out=xt[:, 512:1024], in_=xh_d[0:128, 512:1024])
            else:
                eng.dma_start(out=xt, in_=xh_d[tt * 128:(tt + 1) * 128, :])
            x_tiles.append(xt)

        # ---- constants (two blob DMAs + AP slices) ----
        cbf = const.tile([128, 80], f32, tag="cbf")
        nc.sync.dma_start(out=cbf, in_=cbf_d[:, :])
        cbb = const.tile([128, 4224], bf16, tag="cbb")
        nc.sync.dma_start(out=cbb, in_=cbb_d[:, :])
        bqk_sb = cbf[:, 0:16]
        bg_sb = cbf[:, 16:48]
        bu_sb = cbf[:, 48:80]
        cosq = cbb[:, 0:512]
        msinq = cbb[:, 512:1024]
        cosk = cbb[:, 1024:1792]
        msink = cbb[:, 1792:2560]
        masks = [cbb[:, 2560 + qt * 384:2560 + (qt + 1) * 384] for qt in range(NQT)]
        pshuf = cbb[:, 4096:4224]

        identb = const.tile([128, 128], bf16, tag="identb")
        make_identity(nc, identb)
        ones_row = const.tile([1, 128], bf16, tag="ones_row")
        nc.vector.memset(ones_row, 1.0)
        eps_t = const.tile([128, 1], f32, tag="eps")
        nc.vector.memset(eps_t, EPS)
        bv_sb = const.tile([1, D], bf16, tag="bv")
        nc.sync.dma_start(out=bv_sb, in_=bv_d[:, :])
        bd_sb = const.tile([1, D], bf16, tag="bd")
        nc.sync.dma_start(out=bd_sb, in_=bd_d[:, :])

        # ---- persistent activation pools (LIFO: outermost live longest) ----
        x2_pool = top.enter_context(tc.tile_pool(name="x2", bufs=4))
        y2T_pool = top.enter_context(tc.tile_pool(name="y2T", bufs=8))
        o2_pool = top.enter_context(tc.tile_pool(name="o2", bufs=8))

        def ln_stats(src, tmp_pool):
            """bn stats for one 128-token tile -> mv [128, (mean, var)]."""
            stats = tmp_pool.tile([128, 2, 6], f32, tag="lnstats")
            mv = tmp_pool.tile([128, 2], f32, tag="lnmv")
            for sg in range(2):
                nc.vector.bn_stats(out=stats[:, sg, :], in_=src[:, sg * 512:(sg + 1) * 512])
            nc.vector.bn_aggr(out=mv, in_=stats)
            return mv

        def ln_norm(src, dst, mv, tmp_pool):
            """dst = (src - mean)*rsqrt(var+eps); the affine normalize runs
            on Act via per-partition scale/bias APs."""
            rs = tmp_pool.tile([128, 1], f32, tag="lnrs")
            nc.scalar.activation(out=rs, in_=mv[:, 1:2], func=AF.Sqrt,
                                 bias=eps_t, scale=1.0)
            nc.vector.reciprocal(out=rs, in_=rs)
            nb = tmp_pool.tile([128, 1], f32, tag="lnnb")
            nc.vector.tensor_scalar(out=nb, in0=mv[:, 0:1], scalar1=rs,
                                    scalar2=-1.0, op0=OP.mult, op1=OP.mult)
            nc.scalar.activation(out=dst, in_=src, func=AF.Identity,
                                 bias=nb, scale=rs)

        def layernorm(src, dst, tmp_pool):
            ln_norm(src, dst, ln_stats(src, tmp_pool), tmp_pool)

        yT = []
        qT, kT, v_bf = [], [], []

        qkv_scope = ExitStack()
        yT_pool = qkv_scope.enter_context(tc.tile_pool(name="yT", bufs=8))
        qT_pool = qkv_scope.enter_context(tc.tile_pool(name="qT", bufs=8))
        kT_pool = qkv_scope.enter_context(tc.tile_pool(name="kT", bufs=8))
        vb_pool = qkv_scope.enter_context(tc.tile_pool(name="vb", bufs=6))

        # =========== phase 1a: LN1, y^T (bf16) ===========
        with ExitStack() as ph:
            ln_tmp = ph.enter_context(tc.tile_pool(name="ln_tmp", bufs=6))
            y_pool = ph.enter_context(tc.tile_pool(name="y", bufs=6))
            pst = ph.enter_context(tc.tile_pool(name="pst", bufs=6, space="PSUM"))

            for dtile in range(8):
                yT.append(yT_pool.tile([128, HT], bf16, name="yT", tag="yT"))
            ys = []
            for tt in range(6):
                y = y_pool.tile([128, D], bf16, tag="y")
                layernorm(x_tiles[tt], y, ln_tmp)
                ys.append(y)
            # dtile-outer transposes -> one wide copy per yT tile
            for dtile in range(8):
                pt = pst.tile([128, 6, 128], bf16, tag="pst")
                for tt in range(6):
                    nc.tensor.transpose(pt[:, tt, :],
                                        ys[tt][:, dtile * 128:(dtile + 1) * 128], identb)
                if dtile % 2 == 0:
                    nc.scalar.copy(out=yT[dtile], in_=pt)
                else:
                    nc.vector.tensor_copy(out=yT[dtile], in_=pt)

        # =========== phase 1b: v projection (natural layout, bf16) ===========
        with ExitStack() as ph:
            wv_pool = ph.enter_context(tc.tile_pool(name="wv", bufs=4))
            psv = ph.enter_context(tc.tile_pool(name="psv", bufs=6, space="PSUM"))

            for tt in range(6):
                v_bf.append(vb_pool.tile([128, D], bf16, name="vbf", tag="vbf"))
            for chv in range(2):
                sl = slice(chv * 512, (chv + 1) * 512)
                pv = [psv.tile([128, 512], f32, name="psv", tag="psv") for _ in range(6)]
                for kt in range(8):
                    w = wv_pool.tile([128, 512], bf16, tag="wv")
                    eng = nc.gpsimd if chv == 0 else nc.sync
                    eng.dma_start(out=w, in_=wv_d[chv, kt])
                    for tt in range(6):
                        nc.tensor.matmul(pv[tt], lhsT=yT[kt][:, tt * 128:(tt + 1) * 128],
                                         rhs=w, start=(kt == 0), stop=False)
                for tt in range(6):
                    # += bias via ones-row matmul, closes the accumulation
                    nc.tensor.matmul(pv[tt], lhsT=ones_row, rhs=bv_sb[:, sl],
                                     start=False, stop=True)
                for tt in range(6):
                    if tt % 2 == 0:
                        nc.scalar.copy(out=v_bf[tt][:, sl], in_=pv[tt])
                    else:
                        nc.vector.tensor_copy(out=v_bf[tt][:, sl], in_=pv[tt])

        # ==== phase 2: q/k projections + RoPE software-pipelined with ====
        # ==== attention: proj(mt+1) matmuls fill attn(mt) chain stalls ====
        with ExitStack() as ph:
            wqk_pool = ph.enter_context(tc.tile_pool(name="wqk", bufs=6))
            psb = ph.enter_context(tc.tile_pool(name="psb", bufs=2, space="PSUM"))
            rope_tmp = ph.enter_context(tc.tile_pool(name="rope_tmp", bufs=3))
            at = ph.enter_context(tc.tile_pool(name="at", bufs=8))
            attn_ph = ExitStack()
            psl = attn_ph.enter_context(tc.tile_pool(name="psl", bufs=3, space="PSUM"))
            pstr = attn_ph.enter_context(tc.tile_pool(name="pstr", bufs=2, space="PSUM"))
            pso = attn_ph.enter_context(tc.tile_pool(name="pso", bufs=1, space="PSUM"))

            o2 = [o2_pool.tile([128, CS], bf16, name="o2", tag="o2") for _ in range(8)]

            def rope_pe(dst_slice, src_slice, pr, w):
                nc.tensor.matmul(pr[:, :w], lhsT=pshuf, rhs=src_slice,
                                 start=True, stop=True)

            def proj_chunks(mt):
                """Projection+RoPE for q/k tile mt as a list of emit-closures;
                interleaved between attention stages of tile mt-1."""
                w_q = wqk_pool.tile([128, 8, 128], bf16, tag="wqk")
                nc.sync.dma_start(out=w_q, in_=wq_d[mt])
                w_k = wqk_pool.tile([128, 8, 128], bf16, tag="wqk")
                nc.sync.dma_start(out=w_k, in_=wk_d[mt])
                qt_t = qT_pool.tile([128, CS], bf16, tag="qT")
                kt_t = kT_pool.tile([128, HT], bf16, tag="kT")
                st = {}

                def c0():  # q projection
                    ps = psb.tile([128, CS], f32, tag="psqk")
                    for kt in range(8):
                        nc.tensor.matmul(ps, lhsT=w_q[:, kt, :], rhs=yT[kt][:, WIN:HT],
                                         start=(kt == 0), stop=(kt == 7))
                    qb = rope_tmp.tile([128, HT], bf16, tag="ropesrc")
                    nc.scalar.activation(out=qb[:, :CS], in_=ps, func=AF.Identity,
                                         bias=bqk_sb[:, mt:mt + 1], scale=1.0)
                    st["qb"] = qb

                def c1():  # q rope
                    qb = st["qb"]
                    pr = psb.tile([128, 512], f32, tag="psqk")
                    rope_pe(None, qb[:, :CS], pr, CS)
                    u = rope_tmp.tile([128, HT], bf16, tag="ropeu")
                    nc.vector.tensor_mul(out=u[:, :CS], in0=qb[:, :CS], in1=cosq)
                    t1 = rope_tmp.tile([128, 512], bf16, tag="ropet")
                    nc.vector.tensor_mul(out=t1, in0=pr, in1=msinq)
                    nc.vector.tensor_add(out=qt_t, in0=u[:, :CS], in1=t1)

                def c2():  # k projection half 0
                    kb = rope_tmp.tile([128, HT], bf16, tag="ropesrc")
                    st["kb"] = kb
                    ps = psb.tile([128, 384], f32, tag="psqk")
                    for kt in range(8):
                        nc.tensor.matmul(ps, lhsT=w_k[:, kt, :], rhs=yT[kt][:, 0:384],
                                         start=(kt == 0), stop=(kt == 7))
                    nc.scalar.activation(out=kb[:, 0:384], in_=ps, func=AF.Identity,
                                         bias=bqk_sb[:, 8 + mt:9 + mt], scale=1.0)

                def c3():  # k projection half 1 + k rope
                    kb = st["kb"]
                    ps = psb.tile([128, 384], f32, tag="psqk")
                    for kt in range(8):
                        nc.tensor.matmul(ps, lhsT=w_k[:, kt, :], rhs=yT[kt][:, 384:768],
                                         start=(kt == 0), stop=(kt == 7))
                    nc.scalar.activation(out=kb[:, 384:768], in_=ps, func=AF.Identity,
                                         bias=bqk_sb[:, 8 + mt:9 + mt], scale=1.0)
                    u = rope_tmp.tile([128, HT], bf16, tag="ropeu")
                    nc.vector.tensor_mul(out=u, in0=kb, in1=cosk)
                    for c in range(2):
                        w = 512 if c == 0 else 256
                        sl_ = slice(c * 512, c * 512 + w)
                        pr = psb.tile([128, 512], f32, tag="psqk")
                        rope_pe(None, kb[:, sl_], pr, w)
                        t1 = rope_tmp.tile([128, 512], bf16, tag="ropet")
                        nc.vector.tensor_mul(out=t1[:, :w], in0=pr[:, :w],
                                             in1=msink[:, sl_])
                        nc.vector.tensor_add(out=kt_t[:, sl_], in0=u[:, sl_],
                                             in1=t1[:, :w])

                qT.append(qt_t)
                kT.append(kt_t)
                return [c0, c1, c2, c3]

            def attn_step(mt, qt, filler):
                """One query tile of attention for head-pair mt, with PE filler
                closures injected between dependent stages."""
                ps_l2, E2, sums2 = [], [], []
                for hh in range(2):
                    hr = hh * 64
                    ps_l = psl.tile([128, 384], f32, tag="psl")
                    nc.tensor.matmul(ps_l, lhsT=identb, rhs=masks[qt],
                                     start=True, stop=False)
                    nc.tensor.matmul(ps_l,
                                     lhsT=qT[mt][hr:hr + 64, qt * 128:(qt + 1) * 128],
                                     rhs=kT[mt][hr:hr + 64, qt * 128:qt * 128 + 384],
                                     start=False, stop=True)
                    ps_l2.append(ps_l)
                for hh in range(2):
                    E = at.tile([128, 384], bf16, tag="E")
                    sums = at.tile([128, 1], f32, tag="sums")
                    nc.scalar.activation(out=E, in_=ps_l2[hh], func=AF.Exp,
                                         scale=float(HD) ** -0.5, accum_out=sums)
                    E2.append(E)
                    sums2.append(sums)
                if filler:
                    filler[0]()          # PE filler while exp runs
                ps_t = pstr.tile([128, 2, 384], bf16, tag="pstr")
                for hh in range(2):
                    sums = sums2[hh]
                    nc.vector.reciprocal(out=sums, in_=sums)
                    En = at.tile([128, 384], bf16, tag="En")
                    nc.vector.tensor_scalar_mul(out=En, in0=E2[hh], scalar1=sums)
                    for j in range(3):
                        nc.tensor.transpose(ps_t[:, hh, j * 128:(j + 1) * 128],
                                            En[:, j * 128:(j + 1) * 128], identb)
                ET = at.tile([128, 2, 384], bf16, tag="ET")
                if (mt + qt) % 2 == 0:
                    nc.vector.tensor_copy(out=ET, in_=ps_t)
                else:
                    nc.scalar.copy(out=ET, in_=ps_t)
                if len(filler) > 1:
                    filler[1]()          # PE filler while ET copies drain
                ps_o = pso.tile([128, 128], f32, tag="pso")
                for hh in range(2):
                    h = 2 * mt + hh
                    hr = hh * 64
                    for j in range(3):
                        nc.tensor.matmul(ps_o[hr:hr + 64, :],
                                         lhsT=v_bf[qt + j][:, h * 64:h * 64 + 64],
                                         rhs=ET[:, hh, j * 128:(j + 1) * 128],
                                         start=(j == 0), stop=(j == 2))
                if (mt + qt) % 2 == 0:
                    nc.scalar.copy(out=o2[mt][:, qt * 128:(qt + 1) * 128], in_=ps_o)
                else:
                    nc.vector.tensor_copy(out=o2[mt][:, qt * 128:(qt + 1) * 128],
                                          in_=ps_o)

            # ---- phase 3 resources (shared with phase 2 for interleaving) ----
            wo_pool = ph.enter_context(tc.tile_pool(name="wo", bufs=8))
            ln_tmp2 = ph.enter_context(tc.tile_pool(name="ln_tmp2", bufs=3))
            y2_pool = ph.enter_context(tc.tile_pool(name="y2", bufs=4))
            wo_sb = []
            x2_list = [None] * NQT
            mv2_list = [None] * NQT

            def load_wo():
                for dtile in range(8):
                    w = wo_pool.tile([128, D], bf16, tag="wo")
                    nc.gpsimd.dma_start(out=w, in_=wo_d[dtile])
                    wo_sb.append(w)

            def outproj_chunk(qt):
                def f():
                    x2 = x2_pool.tile([128, D], f32, tag="x2")
                    for ch2 in range(2):
                        sl = slice(ch2 * 512, (ch2 + 1) * 512)
                        ps = psb.tile([128, 512], f32, tag="psqk")
                        for dtile in range(8):
                            nc.tensor.matmul(ps,
                                             lhsT=o2[dtile][:, qt * 128:(qt + 1) * 128],
                                             rhs=wo_sb[dtile][:, sl],
                                             start=(dtile == 0), stop=(dtile == 7))
                        nc.vector.tensor_add(out=x2[:, sl], in0=ps,
                                             in1=x_tiles[2 + qt][:, sl])
                    x2_list[qt] = x2
                    mv2_list[qt] = ln_stats(x2, ln_tmp2)
                return f

            y2_list = [None] * NQT

            def lnfin_chunk(qt):
                def f():
                    y2 = y2_pool.tile([128, D], bf16, tag="y2")
                    ln_norm(x2_list[qt], y2, mv2_list[qt], ln_tmp2)
                    y2_list[qt] = y2
                return f

            chunks = proj_chunks(0)
            for c in chunks:
                c()
            fill_plan = {
                (7, 1): [outproj_chunk(0)],
                (7, 2): [outproj_chunk(1)],
                (7, 3): [outproj_chunk(2)],
            }
            for mt in range(8):
                if mt + 1 < 8:
                    nxt = proj_chunks(mt + 1)
                    if mt + 1 == 2:
                        load_wo()
                for qt in range(NQT):
                    if mt + 1 < 8:
                        filler = [nxt[qt]]
                    else:
                        filler = fill_plan.get((mt, qt), [])
                    attn_step(mt, qt, filler)
            outproj_chunk(NQT - 1)()
            lnfin_chunk(0)()
            lnfin_chunk(1)()
            lnfin_chunk(2)()
            attn_ph.close()

            # ---- y2^T transposes (dtile-outer: one wide copy per dtile) ----
            pst2 = ph.enter_context(tc.tile_pool(name="pst2", bufs=6, space="PSUM"))
            y2T = [y2T_pool.tile([128, CS], bf16, name="y2T", tag="y2T") for _ in range(8)]

            def y2t_transposes(dtiles):
                for dtile in dtiles:
                    pt = pst2.tile([128, 4, 128], bf16, tag="pst2b")
                    for qt in range(NQT):
                        nc.tensor.transpose(pt[:, qt, :],
                                            y2_list[qt][:, dtile * 128:(dtile + 1) * 128],
                                            identb)
                    if dtile % 2 == 0:
                        nc.scalar.copy(out=y2T[dtile], in_=pt)
                    else:
                        nc.vector.tensor_copy(out=y2T[dtile], in_=pt)

            lnfin_chunk(3)()
            y2t_transposes(range(8))

        qkv_scope.close()

        # =========== phase 4: MLP gate/up -> H ===========
        Hs = []
        mlp_scope = ExitStack()
        hh_pool = mlp_scope.enter_context(tc.tile_pool(name="hh", bufs=32))
        with ExitStack() as ph:
            wgu_pool = ph.enter_context(tc.tile_pool(name="wgu", bufs=6))
            psg = ph.enter_context(tc.tile_pool(name="psg", bufs=4, space="PSUM"))
            gu_tmp = ph.enter_context(tc.tile_pool(name="gu_tmp", bufs=6))

            for mt in range(32):
                wg_sb = wgu_pool.tile([128, 8, 128], bf16, tag="wgu")
                nc.sync.dma_start(out=wg_sb, in_=wg_d[mt])
                wu_sb = wgu_pool.tile([128, 8, 128], bf16, tag="wgu")
                nc.sync.dma_start(out=wu_sb, in_=wu_d[mt])
                ps_g = psg.tile([128, CS], f32, tag="psgu")
                ps_u = psg.tile([128, CS], f32, tag="psgu")
                for kt in range(8):
                    nc.tensor.matmul(ps_g, lhsT=wg_sb[:, kt, :], rhs=y2T[kt],
                                     start=(kt == 0), stop=(kt == 7))
                for kt in range(8):
                    nc.tensor.matmul(ps_u, lhsT=wu_sb[:, kt, :], rhs=y2T[kt],
                                     start=(kt == 0), stop=(kt == 7))
                G = gu_tmp.tile([128, CS], bf16, tag="G")
                nc.scalar.activation(out=G, in_=ps_g, func=AF.Identity,
                                     bias=bg_sb[:, mt:mt + 1], scale=1.0)
                U = gu_tmp.tile([128, CS], bf16, tag="U")
                nc.scalar.activation(out=U, in_=ps_u, func=AF.Silu,
                                     bias=bu_sb[:, mt:mt + 1], scale=1.0)
                Ht = hh_pool.tile([128, CS], bf16, tag="hh")
                nc.vector.tensor_mul(out=Ht, in0=G, in1=U)
                Hs.append(Ht)

        # =========== phase 5: down proj + residual + store ===========
        with ExitStack() as ph:
            wd_pool = ph.enter_context(tc.tile_pool(name="wd", bufs=5))
            psd = ph.enter_context(tc.tile_pool(name="psd", bufs=8, space="PSUM"))
            out_pool = ph.enter_context(tc.tile_pool(name="outp", bufs=4))

            ps_d = [psd.tile([128, 512], f32, name="psd", tag="psd") for _ in range(8)]
            for kt in range(31):
                w = wd_pool.tile([128, D], bf16, tag="wd")
                nc.sync.dma_start(out=w, in_=wd_d[kt])
                for tt in range(NQT):
                    for ch3 in range(2):
                        nc.tensor.matmul(ps_d[tt * 2 + ch3],
                                         lhsT=Hs[kt][:, tt * 128:(tt + 1) * 128],
                                         rhs=w[:, ch3 * 512:(ch3 + 1) * 512],
                                         start=(kt == 0), stop=False)
            # final k-tile: close/ship each token tile as soon as it finishes
            w = wd_pool.tile([128, D], bf16, tag="wd")
            nc.sync.dma_start(out=w, in_=wd_d[31])
            for tt in range(NQT):
                ot = out_pool.tile([128, D], f32, tag="outp")
                for ch3 in range(2):
                    sl = slice(ch3 * 512, (ch3 + 1) * 512)
                    nc.tensor.matmul(ps_d[tt * 2 + ch3],
                                     lhsT=Hs[31][:, tt * 128:(tt + 1) * 128],
                                     rhs=w[:, sl], start=False, stop=False)
                    nc.tensor.matmul(ps_d[tt * 2 + ch3], lhsT=ones_row,
                                     rhs=bd_sb[:, sl], start=False, stop=True)
                    nc.vector.tensor_add(out=ot[:, sl], in0=ps_d[tt * 2 + ch3],
                                         in1=x2_list[tt][:, sl])
                nc.sync.dma_start(out=out_d[tt * 128:(tt + 1) * 128, :], in_=ot)
        mlp_scope.close()

    nc.compile()
    return nc


def prep_inputs(x, w_qkv, w_out, g1, b1, g2, b2, w_gate, b_gate, w_up, b_up,
                w_down, b_down):
    """Host-side: fold LN params into weights, pre-tile, build per-core tensors."""
    import ml_dtypes
    f32 = np.float32
    bf16 = ml_dtypes.bfloat16

    def tile_lhsT(w):  # [D, M] -> [mt, p, kt, m]
        Dd, M = w.shape
        return np.ascontiguousarray(
            w.reshape(Dd // 128, 128, M // 128, 128).transpose(2, 1, 0, 3)).astype(bf16)

    wqkv_f = (w_qkv * g1[:, None]).astype(f32)
    bqkv = (b1 @ w_qkv).astype(f32)
    common = {
        "wq": tile_lhsT(wqkv_f[:, :1024]),
        "wk": tile_lhsT(wqkv_f[:, 1024:2048]),
        "wv": np.ascontiguousarray(
            wqkv_f[:, 2048:3072].reshape(8, 128, 2, 512).transpose(2, 0, 1, 3)).astype(bf16),
        "wo": np.ascontiguousarray(w_out.reshape(8, 128, D)).astype(bf16),
        "wg": tile_lhsT((w_gate * g2[:, None]).astype(f32)),
        "wu": tile_lhsT((w_up * g2[:, None]).astype(f32)),
        "wd": np.ascontiguousarray(w_down.reshape(32, 128, D)).astype(bf16),
        "bv": bqkv[2048:].reshape(1, D).astype(bf16),
        "bd": b_down.reshape(1, D).astype(bf16),
    }
    bqk_pt = bqkv[:2048].reshape(16, 128).T          # [p, t]
    bg_pt = (b_gate + b2 @ w_gate).astype(f32).reshape(32, 128).T
    bu_pt = (b_up + b2 @ w_up).astype(f32).reshape(32, 128).T
    common["cbf"] = np.ascontiguousarray(
        np.concatenate([bqk_pt, bg_pt, bu_pt], axis=1)).astype(f32)

    # rotate-half permutation as lhsT: rot[m] = sign(m) * src[sigma(m)]
    # sign folded into the sin tables instead -> pshuf is a pure permutation.
    pshuf = np.zeros((128, 128), f32)
    for m in range(128):
        base = (m // 64) * 64
        r = m % 64
        sig = base + (r + 32) % 64
        pshuf[sig, m] = 1.0
    pshuf = pshuf.astype(bf16)

    half = HD // 2
    inv_freq = 1.0 / (10000.0 ** (np.arange(half, dtype=np.float64) / half))

    def rope_tables(pos):
        t = np.maximum(pos, 0).astype(np.float64)
        freqs = np.outer(t, inv_freq)            # [T, 32]
        emb = np.concatenate([freqs, freqs], 1)  # [T, 64]
        c = np.cos(emb).T.astype(f32)            # [64, T]
        s = np.sin(emb).T.astype(f32)
        # sign-folded sin: rows 0-31 get -sin (they receive -x2), rows 32-63 +sin
        ms = s.copy()
        ms[:32] = -ms[:32]
        return (np.ascontiguousarray(np.vstack([c, c])),
                np.ascontiguousarray(np.vstack([ms, ms])))

    in_maps = []
    for c in range(NCORES):
        b, chunk = c // CH, c % CH
        q0 = chunk * CS
        lo = q0 - WIN
        xh = np.zeros((HT, D), f32)
        src_lo = max(0, lo)
        xh[src_lo - lo:] = x[b, src_lo:q0 + CS]
        xh = xh.astype(bf16)
        pos_k = np.arange(lo, q0 + CS)
        cosk_a, sink_a = rope_tables(pos_k)
        cosq_a = np.ascontiguousarray(cosk_a[:, WIN:]).astype(bf16)
        sinq_a = np.ascontiguousarray(sink_a[:, WIN:]).astype(bf16)
        # mask [qt, r, c]: query i = q0 + qt*128 + r ; key j = lo + qt*128 + cc
        qt_i = np.arange(NQT)[:, None, None]
        r_i = np.arange(128)[None, :, None]
        c_i = np.arange(384)[None, None, :]
        gi = q0 + qt_i * 128 + r_i
        gj = lo + qt_i * 128 + c_i
        valid = (gj <= gi) & (gi - gj <= WIN) & (gj >= 0)
        mask = np.where(valid, 0.0, NEG).astype(bf16)  # [4, 128, 384]
        cbb = np.concatenate(
            [cosq_a, sinq_a, cosk_a.astype(bf16), sink_a.astype(bf16),
             mask.transpose(1, 0, 2).reshape(128, 4 * 384), pshuf], axis=1)
        in_maps.append(dict(common, xh=xh, cbb=np.ascontiguousarray(cbb)))
    return in_maps


_PROG = {}


def kernel(**inputs):
    from concourse.bass_utils import run_bass_kernel_spmd

    inputs = {k: np.asarray(v, dtype=np.float32) for k, v in inputs.items()}
    in_maps = prep_inputs(**inputs)
    if "nc" not in _PROG:
        _PROG["nc"] = build_program()
    nc = _PROG["nc"]
    res = run_bass_kernel_spmd(nc, in_maps, core_ids=list(range(NCORES)))
    out = np.zeros((B, S, D), np.float32)
    for c in range(NCORES):
        b, chunk = c // CH, c % CH
        out[b, chunk * CS:(chunk + 1) * CS] = res.results[c]["out"]
    return out
